# revision 36
# baseline (speedup 1.0000x reference)
import sys

sys.path.insert(0, "/opt/trn_rl_repo")
import numpy as np
import ml_dtypes
import concourse.bass as bass
import concourse.tile as tile
from concourse import bacc, mybir
from concourse.alu_op_type import AluOpType
from concourse.bass_utils import run_bass_kernel_spmd

# Problem constants (nn_EquivGNNEncoder: 2048 graphs x 32 atoms, 3 layers)
B, NA = 2048, 32
N = B * NA                  # 65536 nodes
S_MUL, V_MUL = 32, 16
NCORES = 8
GPC = B // NCORES           # 256 graphs per core
NPC = GPC * NA              # 8192 nodes per core
GPB = 4                     # graphs per block (4*32 = 128 partitions)
NBLK = GPC // GPB           # 64 blocks per core
LAT = 128                   # latent out dim
HID = 256
ND = 112                    # padded node feature dim (s 0:32, v_c at 32+32c:48+32c)

INV_SQRT3 = 1.0 / np.sqrt(3.0)
C_SCALAR = np.float32(1.0 / np.sqrt(48.0))
C_VECTOR = np.float32(np.sqrt(3.0 / 48.0))

F32 = mybir.dt.float32
BF16 = mybir.dt.bfloat16
AF = mybir.ActivationFunctionType

_CACHE = {}


def _ap2(t, poff, psz, chunks, width, cstride):
    """AP over partitions [poff:poff+psz], free = `chunks` blocks of `width`
    elems spaced `cstride` apart (2-level free)."""
    base = t[poff:poff + psz, 0:1]
    return bass.AP(tensor=base.tensor, offset=base.offset,
                   ap=[list(base.ap[0]), [cstride, chunks], [1, width]])


def _build_program():
    nc = bacc.Bacc("TRN2", target_bir_lowering=False, debug=False)

    s0_ap = nc.dram_tensor("s0", [NPC, S_MUL], BF16, kind="ExternalInput").ap()
    posGC_ap = nc.dram_tensor("posGC", [NBLK, 128, 100], F32, kind="ExternalInput").ap()
    posnm_ap = nc.dram_tensor("posnm", [NPC, 3], F32, kind="ExternalInput").ap()
    bd_ap = nc.dram_tensor("bd", [128, 32], F32, kind="ExternalInput").ap()
    # combined transform weights per layer (112-padded feature layout):
    # rows 0:112 = W_A over aggA = ps1[:,0:128] mirror (a_s + av_c, pads zero)
    # rows 112:208 = W_B as_c blocks; rows 208:224 = W4'' (sv)
    wab_ap = nc.dram_tensor("wab", [3, 224, ND], BF16, kind="ExternalInput").ap()
    poolm_ap = nc.dram_tensor("poolm", [128, GPB], BF16, kind="ExternalInput").ap()
    wr1_ap = nc.dram_tensor("wr1", [ND, HID], BF16, kind="ExternalInput").ap()
    br1_ap = nc.dram_tensor("br1", [HID, 1], F32, kind="ExternalInput").ap()
    wr2_ap = nc.dram_tensor("wr2", [HID, LAT], BF16, kind="ExternalInput").ap()
    br2_ap = nc.dram_tensor("br2", [LAT, 1], F32, kind="ExternalInput").ap()
    out_ap = nc.dram_tensor("outfm", [LAT, GPC], F32, kind="ExternalOutput").ap()

    with tile.TileContext(nc) as tc:
        with tc.tile_pool(name="const", bufs=1) as const, \
             tc.tile_pool(name="stage", bufs=26) as stage, \
             tc.tile_pool(name="gmp", bufs=26) as gmp, \
             tc.tile_pool(name="feat", bufs=16) as featp, \
             tc.tile_pool(name="aggt", bufs=10) as aggp, \
             tc.tile_pool(name="work", bufs=26) as work, \
             tc.tile_pool(name="ps1", bufs=3, space="PSUM") as psp1, \
             tc.tile_pool(name="psh", bufs=2, space="PSUM") as psph:

            # --- constants ---
            bd = const.tile([128, 32], F32)
            nc.sync.dma_start(bd[:], bd_ap[:])
            wa = const.tile([ND, 3, ND], BF16)
            nc.sync.dma_start(
                wa[:],
                bass.AP(tensor=wab_ap.tensor, offset=wab_ap.offset,
                        ap=[[ND, ND], [224 * ND, 3], [1, ND]]))
            wb = const.tile([ND, 3, ND], BF16)
            nc.sync.dma_start(
                wb[:],
                bass.AP(tensor=wab_ap.tensor, offset=wab_ap.offset + ND * ND,
                        ap=[[ND, ND], [224 * ND, 3], [1, ND]]))
            poolm = const.tile([128, GPB], BF16)
            nc.sync.dma_start(poolm[:], poolm_ap[:])
            wr1 = const.tile([ND, HID], BF16)
            nc.sync.dma_start(wr1[:], wr1_ap[:])
            wr2a = const.tile([128, LAT], BF16)
            nc.sync.dma_start(wr2a[:], wr2_ap[0:128, :])
            wr2b = const.tile([128, LAT], BF16)
            nc.sync.dma_start(wr2b[:], wr2_ap[128:256, :])
            br1a = const.tile([128, 1], F32)
            nc.sync.dma_start(br1a[:], br1_ap[0:128, :])
            br1b = const.tile([128, 1], F32)
            nc.sync.dma_start(br1b[:], br1_ap[128:256, :])
            br2 = const.tile([LAT, 1], F32)
            nc.sync.dma_start(br2[:], br2_ap[:])
            epsb = const.tile([128, 1], F32)
            nc.vector.memset(epsb[:], 1e-12)

            xfm = const.tile([ND, GPC], BF16)

            gmbuf_count = [0]

            def emit_load_gm(b):
                fc = stage.tile([128, 100], F32, tag="fall")
                nc.sync.dma_start(
                    fc[:],
                    bass.AP(tensor=posGC_ap.tensor,
                            offset=posGC_ap.offset + b * 128 * 100,
                            ap=[[100, 128], [1, 100]]))
                pos3 = fc[:, 96:99]

                diffc = work.tile([128, 96], F32, tag="diff")
                for c in range(3):
                    nc.vector.tensor_scalar(
                        diffc[:, 32 * c:32 * (c + 1)], fc[:, 32 * c:32 * (c + 1)],
                        fc[:, 96 + c:97 + c], None, AluOpType.subtract)
                sqc = work.tile([128, 96], F32, tag="sq")
                nc.scalar.activation(sqc[:], diffc[:], AF.Square)
                d2c = work.tile([128, 32], F32, tag="d2")
                nc.vector.tensor_reduce(
                    d2c[:],
                    bass.AP(tensor=sqc.tensor, offset=sqc.offset,
                            ap=[list(sqc[:, 0:1].ap[0]), [1, 32], [32, 3]]),
                    mybir.AxisListType.X, AluOpType.add)

                rs = work.tile([128, 32], F32, tag="rs")
                nc.scalar.activation(rs[:], d2c[:], AF.Abs_reciprocal_sqrt,
                                     bias=epsb[:], scale=float(1.0 / 3.0))

                # compact mask: (d2 <= 25) * bd_noself
                cgm = work.tile([128, 32], BF16, tag="cgm")
                nc.vector.scalar_tensor_tensor(
                    cgm[:], d2c[:], 25.0, bd[:], AluOpType.is_le, AluOpType.mult)
                ga = work.tile([128, 32], F32, tag="ga")
                nc.gpsimd.tensor_mul(ga[:], rs[:], cgm[:])

                gm = gmp.tile([128, 512], BF16, tag="gm")
                if gmbuf_count[0] < 26:
                    gmbuf_count[0] += 1
                    nc.vector.memset(gm[:], 0.0)
                for g in range(GPB):
                    gbase = gm[32 * g:32 * (g + 1), 0:1]
                    dbase = diffc[32 * g:32 * (g + 1), 0:1]
                    abase = ga[32 * g:32 * (g + 1), 0:1]
                    # sh blocks written straight into the diagonal slots
                    nc.gpsimd.tensor_mul(
                        bass.AP(tensor=gm.tensor, offset=gbase.offset + 128 + 32 * g,
                                ap=[list(gbase.ap[0]), [128, 3], [1, 32]]),
                        bass.AP(tensor=diffc.tensor, offset=dbase.offset,
                                ap=[list(dbase.ap[0]), [32, 3], [1, 32]]),
                        bass.AP(tensor=ga.tensor, offset=abase.offset,
                                ap=[list(abase.ap[0]), [0, 3], [1, 32]]))
                    nc.gpsimd.tensor_copy(
                        gm[32 * g:32 * (g + 1), 32 * g:32 * (g + 1)],
                        cgm[32 * g:32 * (g + 1), :])
                return gm

            def emit_layer_front(l, gms, featpr, s0pr):
                # paired PSUM tile: block i at cols 512*i (2 banks)
                ps1 = psp1.tile([ND, 1024], F32, tag="ps1")
                for i in range(2):
                    if l == 0:
                        nc.tensor.matmul(ps1[0:32, 512 * i:512 * (i + 1)],
                                         s0pr[:, 32 * i:32 * (i + 1)],
                                         gms[i][:], start=True, stop=True)
                    else:
                        nc.tensor.matmul(ps1[0:112, 512 * i:512 * (i + 1)],
                                         featpr[:, 112 * i:112 * (i + 1)],
                                         gms[i][:], start=True, stop=True)

                # paired agg tile: block i at cols 256*i, [A 0:128 | B 128:256]
                agg = aggp.tile([ND, 512], BF16, tag="agg")
                if l == 0:
                    # merged A-half + as_0 copy: rows 0:32, 256-wide chunks
                    nc.scalar.copy(_ap2(agg, 0, 32, 2, 256, 256),
                                   _ap2(ps1, 0, 32, 2, 256, 512))
                else:
                    nc.scalar.copy(_ap2(agg, 0, 112, 2, 128, 256),
                                   _ap2(ps1, 0, 112, 2, 128, 512))
                    nc.scalar.copy(
                        bass.AP(tensor=agg.tensor, offset=agg[0:32, 0:1].offset + 128,
                                ap=[list(agg[0:32, 0:1].ap[0]), [256, 2], [1, 128]]),
                        bass.AP(tensor=ps1.tensor, offset=ps1[0:32, 0:1].offset + 128,
                                ap=[list(ps1[0:32, 0:1].ap[0]), [512, 2], [1, 128]]))
                nc.vector.tensor_copy(
                    bass.AP(tensor=agg.tensor, offset=agg[32:64, 0:1].offset + 128,
                            ap=[list(agg[32:64, 0:1].ap[0]), [256, 2], [1, 128]]),
                    bass.AP(tensor=ps1.tensor, offset=ps1[0:32, 0:1].offset + 256,
                            ap=[list(ps1[0:32, 0:1].ap[0]), [512, 2], [1, 128]]))
                nc.scalar.copy(
                    bass.AP(tensor=agg.tensor, offset=agg[64:96, 0:1].offset + 128,
                            ap=[list(agg[64:96, 0:1].ap[0]), [256, 2], [1, 128]]),
                    bass.AP(tensor=ps1.tensor, offset=ps1[0:32, 0:1].offset + 384,
                            ap=[list(ps1[0:32, 0:1].ap[0]), [512, 2], [1, 128]]))
                if l != 0:
                    svt = work.tile([16, 256], F32, tag="svt")
                    nc.scalar.copy(svt[:],
                                   bass.AP(tensor=ps1.tensor,
                                           offset=ps1[32:48, 0:1].offset + 128,
                                           ap=[list(ps1[32:48, 0:1].ap[0]),
                                               [512, 2], [1, 128]]))
                    svt2 = work.tile([16, 256], F32, tag="svt2")
                    nc.vector.tensor_add(
                        svt2[:], svt[:],
                        bass.AP(tensor=ps1.tensor,
                                offset=ps1[64:80, 0:1].offset + 256,
                                ap=[list(ps1[64:80, 0:1].ap[0]), [512, 2], [1, 128]]))
                    nc.vector.tensor_add(
                        bass.AP(tensor=agg.tensor,
                                offset=agg[96:112, 0:1].offset + 128,
                                ap=[list(agg[96:112, 0:1].ap[0]), [256, 2], [1, 128]]),
                        svt2[:],
                        bass.AP(tensor=ps1.tensor,
                                offset=ps1[96:112, 0:1].offset + 384,
                                ap=[list(ps1[96:112, 0:1].ap[0]), [512, 2], [1, 128]]))
                return agg, ps1

            def emit_layer_back(l, agg, featpr, s0pr):
                ka = 32 if l == 0 else 112
                kb = 96 if l == 0 else 112
                ps_h = psph.tile([128, 512], F32, tag="psh")
                for i in range(2):
                    nc.tensor.matmul(ps_h[:, ND * i:ND * (i + 1)],
                                     agg[0:ka, 256 * i:256 * i + 128],
                                     wa[0:ka, l, :], start=True, stop=False)
                    nc.tensor.matmul(ps_h[:, ND * i:ND * (i + 1)],
                                     agg[0:kb, 256 * i + 128:256 * i + 256],
                                     wb[0:kb, l, :], start=False, stop=True)

                featn = featp.tile([128, 2 * ND], BF16, tag="feat")
                if l == 0:
                    # s part: residual with s0; v part: plain relu
                    nc.vector.scalar_tensor_tensor(
                        bass.AP(tensor=featn.tensor, offset=featn[:, 0:1].offset,
                                ap=[list(featn[:, 0:1].ap[0]), [ND, 2], [1, 32]]),
                        bass.AP(tensor=ps_h.tensor, offset=ps_h[:, 0:1].offset,
                                ap=[list(ps_h[:, 0:1].ap[0]), [ND, 2], [1, 32]]),
                        0.0,
                        s0pr[:],
                        AluOpType.max, AluOpType.add)
                    nc.scalar.activation(
                        bass.AP(tensor=featn.tensor, offset=featn[:, 0:1].offset + 32,
                                ap=[list(featn[:, 0:1].ap[0]), [ND, 2], [1, 80]]),
                        bass.AP(tensor=ps_h.tensor, offset=ps_h[:, 0:1].offset + 32,
                                ap=[list(ps_h[:, 0:1].ap[0]), [ND, 2], [1, 80]]),
                        AF.Relu)
                else:
                    nc.vector.scalar_tensor_tensor(
                        featn[:], ps_h[:, 0:2 * ND], 0.0, featpr[:],
                        AluOpType.max, AluOpType.add)
                return featn

            def emit_pool_pair(pr, featpr, ps1):
                for i in range(2):
                    nc.tensor.matmul(ps1[0:ND, 512 * i:512 * i + GPB],
                                     featpr[:, ND * i:ND * (i + 1)],
                                     poolm[:], start=True, stop=True)
                nc.vector.tensor_copy(
                    bass.AP(tensor=xfm.tensor,
                            offset=xfm[:, 0:1].offset + 2 * GPB * pr,
                            ap=[list(xfm[:, 0:1].ap[0]), [GPB, 2], [1, GPB]]),
                    bass.AP(tensor=ps1.tensor, offset=ps1[0:ND, 0:1].offset,
                            ap=[list(ps1[0:ND, 0:1].ap[0]), [512, 2], [1, GPB]]))

            # process blocks in pairs with depth-2 geometry prefetch so
            # engine queues interleave independent work
            NP = NBLK // 2

            def emit_load_pair(pr):
                b0, b1 = 2 * pr, 2 * pr + 1
                gms = [emit_load_gm(b0), emit_load_gm(b1)]
                s0pr = stage.tile([128, 64], BF16, tag="s0b")
                nc.sync.dma_start(s0pr[:, 0:32], s0_ap[b0 * 128:(b0 + 1) * 128, :])
                nc.sync.dma_start(s0pr[:, 32:64], s0_ap[b1 * 128:(b1 + 1) * 128, :])
                return gms, s0pr

            GW = 4  # pairs interleaved per group
            pending = [emit_load_pair(k) for k in range(GW)]
            loaded = GW
            base = 0
            while base < NP:
                gn = min(GW, NP - base)
                while loaded < min(base + 2 * gn, NP):
                    pending.append(emit_load_pair(loaded))
                    loaded += 1
                ctx = [list(pending.pop(0)) + [None, None] for _ in range(gn)]
                for l in range(3):
                    aggs = []
                    for i in range(gn):
                        gms, s0pr, ft, _ = ctx[i]
                        agg_i, lps1 = emit_layer_front(l, gms, ft, s0pr)
                        aggs.append(agg_i)
                        ctx[i][3] = lps1
                    for i in range(gn):
                        gms, s0pr, ft, _ = ctx[i]
                        ctx[i][2] = emit_layer_back(l, aggs[i], ft, s0pr)
                for i in range(gn):
                    emit_pool_pair(base + i, ctx[i][2], ctx[i][3])
                base += gn

            # --- readout MLP: relu(x @ Wr1 + br1) @ Wr2 + br2, feature-major ---
            ps_h1 = psph.tile([128, GPC], F32, tag="psh")
            ps_h2 = psph.tile([128, GPC], F32, tag="psh")
            nc.tensor.matmul(ps_h1[:], wr1[:, 0:128],
                             xfm[:], start=True, stop=True)
            nc.tensor.matmul(ps_h2[:], wr1[:, 128:256],
                             xfm[:], start=True, stop=True)
            hid1 = work.tile([128, GPC], BF16, tag="hid1")
            hid2 = work.tile([128, GPC], BF16, tag="hid2")
            nc.vector.tensor_scalar(hid1[:], ps_h1[:], br1a[:], 0.0,
                                    AluOpType.add, AluOpType.max)
            nc.vector.tensor_scalar(hid2[:], ps_h2[:], br1b[:], 0.0,
                                    AluOpType.add, AluOpType.max)
            ps_o = psp1.tile([LAT, GPC], F32, tag="ps1")
            nc.tensor.matmul(ps_o[:], wr2a[:],
                             hid1[:], start=True, stop=False)
            nc.tensor.matmul(ps_o[:], wr2b[:],
                             hid2[:], start=False, stop=True)
            outt = work.tile([LAT, GPC], F32, tag="outt")
            nc.vector.tensor_scalar(outt[:], ps_o[:], br2[:], None, AluOpType.add)
            nc.sync.dma_start(out_ap[:], outt[:])

    nc.compile()
    return nc


def kernel(pos, emb, W_s2n, W1, W2, W3, W4, Ws, Wv, Wr1, br1, Wr2, br2,
           z, batch, edge_index, num_graphs):
    pos = np.asarray(pos, dtype=np.float32)
    z = np.asarray(z)
    emb = np.asarray(emb, dtype=np.float32)
    W_s2n = np.asarray(W_s2n, dtype=np.float32)
    W1 = np.asarray(W1, dtype=np.float32); W2 = np.asarray(W2, dtype=np.float32)
    W3 = np.asarray(W3, dtype=np.float32); W4 = np.asarray(W4, dtype=np.float32)
    Ws = np.asarray(Ws, dtype=np.float32); Wv = np.asarray(Wv, dtype=np.float32)
    Wr1 = np.asarray(Wr1, dtype=np.float32); br1 = np.asarray(br1, dtype=np.float32)
    Wr2 = np.asarray(Wr2, dtype=np.float32); br2 = np.asarray(br2, dtype=np.float32)

    # host prep: embedding lookup folded with input linear
    EW = (emb @ W_s2n) * np.float32(1.0 / np.sqrt(S_MUL))     # [100, 32]
    s0 = EW[z].astype(ml_dtypes.bfloat16)                     # [N, 32]

    # combined transform weights, norm constants folded in
    cs = C_SCALAR * np.float32(1.0 / np.sqrt(S_MUL))
    csb = C_SCALAR * np.float32(INV_SQRT3 / np.sqrt(S_MUL))
    cv = C_VECTOR * np.float32(INV_SQRT3 / np.sqrt(V_MUL))
    wab = np.zeros((3, 224, ND), np.float32)
    for l in range(3):
        wab[l, 0:32, 0:32] = cs * (W1[l] @ Ws[l])
        w3 = cv * (W3[l] @ Wv[l])
        w2 = cv * (W2[l] @ Wv[l])
        for c in range(3):
            wab[l, 32 + 32 * c:48 + 32 * c, 32 + 32 * c:48 + 32 * c] = w3
            wab[l, 112 + 32 * c:144 + 32 * c, 32 + 32 * c:48 + 32 * c] = w2
        wab[l, 208:224, 0:32] = csb * (W4[l] @ Ws[l])
    wab = wab.astype(ml_dtypes.bfloat16)

    # readout weights in padded feature layout: row 32+32c+u <-> v[u, c]
    wr1p = np.zeros((ND, HID), np.float32)
    wr1p[0:32] = Wr1[0:32]
    for c in range(3):
        for u in range(V_MUL):
            wr1p[32 + 32 * c + u] = Wr1[32 + 3 * u + c]
    wr1p = wr1p.astype(ml_dtypes.bfloat16)

    # compact same-graph mask: node i vs its graph's 32 atoms; self excluded
    bdm = np.ones((128, NA), np.float32)
    for i in range(128):
        bdm[i, i % NA] = 0.0
    poolm = np.zeros((128, GPB), np.float32)
    for g in range(GPB):
        poolm[g * NA:(g + 1) * NA, g] = 1.0
    poolm = poolm.astype(ml_dtypes.bfloat16)

    if "nc" not in _CACHE:
        _CACHE["nc"] = _build_program()
    nc = _CACHE["nc"]

    in_maps = []
    for c in range(NCORES):
        psl = pos[c * NPC:(c + 1) * NPC]                       # [8192, 3]
        # posGC[b, 32g+i, 32c+j] = pos of atom j (component c) of node i's graph
        pg = psl.reshape(NBLK, GPB, NA, 3).transpose(0, 1, 3, 2)   # [b, g, c, j]
        posGC = np.zeros((NBLK, 128, 100), np.float32)
        posGC[:, :, 0:96] = np.broadcast_to(
            pg[:, :, None, :, :], (NBLK, GPB, NA, 3, NA)).reshape(NBLK, 128, 96)
        posGC[:, :, 96:99] = psl.reshape(NBLK, 128, 3)
        posGC = np.ascontiguousarray(posGC)
        in_maps.append(dict(
            s0=np.ascontiguousarray(s0[c * NPC:(c + 1) * NPC]),
            posGC=posGC,
            posnm=np.ascontiguousarray(psl),
            bd=bdm, wab=wab, poolm=poolm,
            wr1=wr1p, br1=br1.reshape(HID, 1),
            wr2=Wr2.astype(ml_dtypes.bfloat16), br2=br2.reshape(LAT, 1),
        ))

    res = run_bass_kernel_spmd(nc, in_maps, core_ids=list(range(NCORES)))
    out = np.empty((B, LAT), np.float32)
    for c in range(NCORES):
        out[c * GPC:(c + 1) * GPC] = res.results[c]["outfm"].T
    return out


# revision 37
# speedup vs baseline: 1.2845x; 1.2845x over previous
import sys

sys.path.insert(0, "/opt/trn_rl_repo")
import numpy as np
import ml_dtypes
import concourse.bass as bass
import concourse.tile as tile
from concourse import bacc, mybir
from concourse.alu_op_type import AluOpType
from concourse.bass_utils import run_bass_kernel_spmd

# Problem constants (nn_EquivGNNEncoder: 2048 graphs x 32 atoms, 3 layers)
B, NA = 2048, 32
N = B * NA                  # 65536 nodes
S_MUL, V_MUL = 32, 16
NCORES = 8
GPC = B // NCORES           # 256 graphs per core
NPC = GPC * NA              # 8192 nodes per core
GPB = 4                     # graphs per block (4*32 = 128 partitions)
NBLK = GPC // GPB           # 64 blocks per core
LAT = 128                   # latent out dim
HID = 256
ND = 112                    # padded node feature dim (s 0:32, v_c at 32+32c:48+32c)

INV_SQRT3 = 1.0 / np.sqrt(3.0)
C_SCALAR = np.float32(1.0 / np.sqrt(48.0))
C_VECTOR = np.float32(np.sqrt(3.0 / 48.0))

F32 = mybir.dt.float32
BF16 = mybir.dt.bfloat16
AF = mybir.ActivationFunctionType

_CACHE = {}


def _ap2(t, poff, psz, chunks, width, cstride):
    """AP over partitions [poff:poff+psz], free = `chunks` blocks of `width`
    elems spaced `cstride` apart (2-level free)."""
    base = t[poff:poff + psz, 0:1]
    return bass.AP(tensor=base.tensor, offset=base.offset,
                   ap=[list(base.ap[0]), [cstride, chunks], [1, width]])


def _build_program():
    nc = bacc.Bacc("TRN2", target_bir_lowering=False, debug=False)

    s0_ap = nc.dram_tensor("s0", [NPC, S_MUL], BF16, kind="ExternalInput").ap()
    posGC_ap = nc.dram_tensor("posGC", [NBLK, 128, 96], F32, kind="ExternalInput").ap()
    posnm_ap = nc.dram_tensor("posnm", [NPC, 3], F32, kind="ExternalInput").ap()
    bd_ap = nc.dram_tensor("bd", [128, 32], F32, kind="ExternalInput").ap()
    # combined transform weights per layer (112-padded feature layout):
    # rows 0:112 = W_A over aggA = ps1[:,0:128] mirror (a_s + av_c, pads zero)
    # rows 112:208 = W_B as_c blocks; rows 208:224 = W4'' (sv)
    wab_ap = nc.dram_tensor("wab", [3, 224, ND], BF16, kind="ExternalInput").ap()
    poolm_ap = nc.dram_tensor("poolm", [128, GPB], BF16, kind="ExternalInput").ap()
    wr1_ap = nc.dram_tensor("wr1", [ND, HID], BF16, kind="ExternalInput").ap()
    br1_ap = nc.dram_tensor("br1", [HID, 1], F32, kind="ExternalInput").ap()
    wr2_ap = nc.dram_tensor("wr2", [HID, LAT], BF16, kind="ExternalInput").ap()
    br2_ap = nc.dram_tensor("br2", [LAT, 1], F32, kind="ExternalInput").ap()
    out_ap = nc.dram_tensor("outfm", [LAT, GPC], F32, kind="ExternalOutput").ap()

    with tile.TileContext(nc) as tc:
        with tc.tile_pool(name="const", bufs=1) as const, \
             tc.tile_pool(name="stage", bufs=26) as stage, \
             tc.tile_pool(name="gmp", bufs=26) as gmp, \
             tc.tile_pool(name="feat", bufs=14) as featp, \
             tc.tile_pool(name="aggt", bufs=8) as aggp, \
             tc.tile_pool(name="work", bufs=26) as work, \
             tc.tile_pool(name="ps1", bufs=3, space="PSUM") as psp1, \
             tc.tile_pool(name="psh", bufs=2, space="PSUM") as psph:

            # --- constants ---
            bd = const.tile([128, 32], F32)
            nc.sync.dma_start(bd[:], bd_ap[:])
            wa = const.tile([ND, 3, ND], BF16)
            nc.sync.dma_start(
                wa[:],
                bass.AP(tensor=wab_ap.tensor, offset=wab_ap.offset,
                        ap=[[ND, ND], [224 * ND, 3], [1, ND]]))
            wb = const.tile([ND, 3, ND], BF16)
            nc.sync.dma_start(
                wb[:],
                bass.AP(tensor=wab_ap.tensor, offset=wab_ap.offset + ND * ND,
                        ap=[[ND, ND], [224 * ND, 3], [1, ND]]))
            poolm = const.tile([128, GPB], BF16)
            nc.sync.dma_start(poolm[:], poolm_ap[:])
            wr1 = const.tile([ND, HID], BF16)
            nc.sync.dma_start(wr1[:], wr1_ap[:])
            wr2a = const.tile([128, LAT], BF16)
            nc.sync.dma_start(wr2a[:], wr2_ap[0:128, :])
            wr2b = const.tile([128, LAT], BF16)
            nc.sync.dma_start(wr2b[:], wr2_ap[128:256, :])
            br1a = const.tile([128, 1], F32)
            nc.sync.dma_start(br1a[:], br1_ap[0:128, :])
            br1b = const.tile([128, 1], F32)
            nc.sync.dma_start(br1b[:], br1_ap[128:256, :])
            br2 = const.tile([LAT, 1], F32)
            nc.sync.dma_start(br2[:], br2_ap[:])
            epsb = const.tile([128, 1], F32)
            nc.vector.memset(epsb[:], 1e-12)

            xfm = const.tile([ND, GPC], BF16)

            gmbuf_count = [0]

            def emit_load_gm(b):
                fc = stage.tile([128, 96], F32, tag="fall")
                nc.sync.dma_start(
                    fc[:],
                    bass.AP(tensor=posGC_ap.tensor,
                            offset=posGC_ap.offset + b * 128 * 96,
                            ap=[[96, 128], [1, 96]]))
                pos3 = stage.tile([128, 3], F32, tag="pos3")
                nc.sync.dma_start(pos3[:], posnm_ap[b * 128:(b + 1) * 128, :])

                diffc = work.tile([128, 96], F32, tag="diff")
                for c in range(3):
                    nc.vector.tensor_scalar(
                        diffc[:, 32 * c:32 * (c + 1)], fc[:, 32 * c:32 * (c + 1)],
                        pos3[:, c:c + 1], None, AluOpType.subtract)
                sqc = work.tile([128, 96], F32, tag="sq")
                nc.scalar.activation(sqc[:], diffc[:], AF.Square)
                d2c = work.tile([128, 32], F32, tag="d2")
                nc.vector.tensor_reduce(
                    d2c[:],
                    bass.AP(tensor=sqc.tensor, offset=sqc.offset,
                            ap=[list(sqc[:, 0:1].ap[0]), [1, 32], [32, 3]]),
                    mybir.AxisListType.X, AluOpType.add)

                rs = work.tile([128, 32], F32, tag="rs")
                nc.scalar.activation(rs[:], d2c[:], AF.Abs_reciprocal_sqrt,
                                     bias=epsb[:], scale=float(1.0 / 3.0))

                # compact mask: (d2 <= 25) * bd_noself
                cgm = work.tile([128, 32], BF16, tag="cgm")
                nc.vector.scalar_tensor_tensor(
                    cgm[:], d2c[:], 25.0, bd[:], AluOpType.is_le, AluOpType.mult)
                ga = work.tile([128, 32], F32, tag="ga")
                nc.gpsimd.tensor_mul(ga[:], rs[:], cgm[:])

                gm = gmp.tile([128, 512], BF16, tag="gm")
                if gmbuf_count[0] < 26:
                    gmbuf_count[0] += 1
                    nc.vector.memset(gm[:], 0.0)
                for g in range(GPB):
                    gbase = gm[32 * g:32 * (g + 1), 0:1]
                    dbase = diffc[32 * g:32 * (g + 1), 0:1]
                    abase = ga[32 * g:32 * (g + 1), 0:1]
                    # sh blocks written straight into the diagonal slots
                    nc.gpsimd.tensor_mul(
                        bass.AP(tensor=gm.tensor, offset=gbase.offset + 128 + 32 * g,
                                ap=[list(gbase.ap[0]), [128, 3], [1, 32]]),
                        bass.AP(tensor=diffc.tensor, offset=dbase.offset,
                                ap=[list(dbase.ap[0]), [32, 3], [1, 32]]),
                        bass.AP(tensor=ga.tensor, offset=abase.offset,
                                ap=[list(abase.ap[0]), [0, 3], [1, 32]]))
                    nc.gpsimd.tensor_copy(
                        gm[32 * g:32 * (g + 1), 32 * g:32 * (g + 1)],
                        cgm[32 * g:32 * (g + 1), :])
                return gm

            def emit_layer_front(l, gms, featpr, s0pr):
                # paired PSUM tile: block i at cols 512*i (2 banks)
                ps1 = psp1.tile([ND, 1024], F32, tag="ps1")
                for i in range(2):
                    if l == 0:
                        nc.tensor.matmul(ps1[0:32, 512 * i:512 * (i + 1)],
                                         s0pr[:, 32 * i:32 * (i + 1)],
                                         gms[i][:], start=True, stop=True)
                    else:
                        nc.tensor.matmul(ps1[0:112, 512 * i:512 * (i + 1)],
                                         featpr[:, 112 * i:112 * (i + 1)],
                                         gms[i][:], start=True, stop=True)

                # paired agg tile: block i at cols 256*i, [A 0:128 | B 128:256]
                agg = aggp.tile([ND, 512], BF16, tag="agg")
                if l == 0:
                    # merged A-half + as_0 copy: rows 0:32, 256-wide chunks
                    nc.scalar.copy(_ap2(agg, 0, 32, 2, 256, 256),
                                   _ap2(ps1, 0, 32, 2, 256, 512))
                else:
                    nc.scalar.copy(_ap2(agg, 0, 112, 2, 128, 256),
                                   _ap2(ps1, 0, 112, 2, 128, 512))
                    nc.scalar.copy(
                        bass.AP(tensor=agg.tensor, offset=agg[0:32, 0:1].offset + 128,
                                ap=[list(agg[0:32, 0:1].ap[0]), [256, 2], [1, 128]]),
                        bass.AP(tensor=ps1.tensor, offset=ps1[0:32, 0:1].offset + 128,
                                ap=[list(ps1[0:32, 0:1].ap[0]), [512, 2], [1, 128]]))
                nc.vector.tensor_copy(
                    bass.AP(tensor=agg.tensor, offset=agg[32:64, 0:1].offset + 128,
                            ap=[list(agg[32:64, 0:1].ap[0]), [256, 2], [1, 128]]),
                    bass.AP(tensor=ps1.tensor, offset=ps1[0:32, 0:1].offset + 256,
                            ap=[list(ps1[0:32, 0:1].ap[0]), [512, 2], [1, 128]]))
                nc.scalar.copy(
                    bass.AP(tensor=agg.tensor, offset=agg[64:96, 0:1].offset + 128,
                            ap=[list(agg[64:96, 0:1].ap[0]), [256, 2], [1, 128]]),
                    bass.AP(tensor=ps1.tensor, offset=ps1[0:32, 0:1].offset + 384,
                            ap=[list(ps1[0:32, 0:1].ap[0]), [512, 2], [1, 128]]))
                if l != 0:
                    svt = work.tile([16, 256], F32, tag="svt")
                    nc.scalar.copy(svt[:],
                                   bass.AP(tensor=ps1.tensor,
                                           offset=ps1[32:48, 0:1].offset + 128,
                                           ap=[list(ps1[32:48, 0:1].ap[0]),
                                               [512, 2], [1, 128]]))
                    svt2 = work.tile([16, 256], F32, tag="svt2")
                    nc.vector.tensor_add(
                        svt2[:], svt[:],
                        bass.AP(tensor=ps1.tensor,
                                offset=ps1[64:80, 0:1].offset + 256,
                                ap=[list(ps1[64:80, 0:1].ap[0]), [512, 2], [1, 128]]))
                    nc.vector.tensor_add(
                        bass.AP(tensor=agg.tensor,
                                offset=agg[96:112, 0:1].offset + 128,
                                ap=[list(agg[96:112, 0:1].ap[0]), [256, 2], [1, 128]]),
                        svt2[:],
                        bass.AP(tensor=ps1.tensor,
                                offset=ps1[96:112, 0:1].offset + 384,
                                ap=[list(ps1[96:112, 0:1].ap[0]), [512, 2], [1, 128]]))
                return agg, ps1

            def emit_layer_back(l, agg, featpr, s0pr):
                ka = 32 if l == 0 else 112
                kb = 96 if l == 0 else 112
                ps_h = psph.tile([128, 512], F32, tag="psh")
                for i in range(2):
                    nc.tensor.matmul(ps_h[:, ND * i:ND * (i + 1)],
                                     agg[0:ka, 256 * i:256 * i + 128],
                                     wa[0:ka, l, :], start=True, stop=False)
                    nc.tensor.matmul(ps_h[:, ND * i:ND * (i + 1)],
                                     agg[0:kb, 256 * i + 128:256 * i + 256],
                                     wb[0:kb, l, :], start=False, stop=True)

                featn = featp.tile([128, 2 * ND], BF16, tag="feat")
                if l == 0:
                    # s part: residual with s0; v part: plain relu
                    nc.vector.scalar_tensor_tensor(
                        bass.AP(tensor=featn.tensor, offset=featn[:, 0:1].offset,
                                ap=[list(featn[:, 0:1].ap[0]), [ND, 2], [1, 32]]),
                        bass.AP(tensor=ps_h.tensor, offset=ps_h[:, 0:1].offset,
                                ap=[list(ps_h[:, 0:1].ap[0]), [ND, 2], [1, 32]]),
                        0.0,
                        s0pr[:],
                        AluOpType.max, AluOpType.add)
                    nc.scalar.activation(
                        bass.AP(tensor=featn.tensor, offset=featn[:, 0:1].offset + 32,
                                ap=[list(featn[:, 0:1].ap[0]), [ND, 2], [1, 80]]),
                        bass.AP(tensor=ps_h.tensor, offset=ps_h[:, 0:1].offset + 32,
                                ap=[list(ps_h[:, 0:1].ap[0]), [ND, 2], [1, 80]]),
                        AF.Relu)
                else:
                    nc.vector.scalar_tensor_tensor(
                        featn[:], ps_h[:, 0:2 * ND], 0.0, featpr[:],
                        AluOpType.max, AluOpType.add)
                return featn

            def emit_pool_pair(pr, featpr, ps1):
                for i in range(2):
                    nc.tensor.matmul(ps1[0:ND, 512 * i:512 * i + GPB],
                                     featpr[:, ND * i:ND * (i + 1)],
                                     poolm[:], start=True, stop=True)
                nc.vector.tensor_copy(
                    bass.AP(tensor=xfm.tensor,
                            offset=xfm[:, 0:1].offset + 2 * GPB * pr,
                            ap=[list(xfm[:, 0:1].ap[0]), [GPB, 2], [1, GPB]]),
                    bass.AP(tensor=ps1.tensor, offset=ps1[0:ND, 0:1].offset,
                            ap=[list(ps1[0:ND, 0:1].ap[0]), [512, 2], [1, GPB]]))

            # process blocks in pairs with depth-2 geometry prefetch so
            # engine queues interleave independent work
            NP = NBLK // 2

            def emit_load_pair(pr):
                b0, b1 = 2 * pr, 2 * pr + 1
                gms = [emit_load_gm(b0), emit_load_gm(b1)]
                s0pr = stage.tile([128, 64], BF16, tag="s0b")
                nc.sync.dma_start(s0pr[:, 0:32], s0_ap[b0 * 128:(b0 + 1) * 128, :])
                nc.sync.dma_start(s0pr[:, 32:64], s0_ap[b1 * 128:(b1 + 1) * 128, :])
                return gms, s0pr

            GW = 4  # pairs interleaved per group
            pending = [emit_load_pair(k) for k in range(GW)]
            loaded = GW
            base = 0
            while base < NP:
                gn = min(GW, NP - base)
                while loaded < min(base + 2 * gn, NP):
                    pending.append(emit_load_pair(loaded))
                    loaded += 1
                ctx = [list(pending.pop(0)) + [None, None] for _ in range(gn)]
                for l in range(3):
                    aggs = []
                    for i in range(gn):
                        gms, s0pr, ft, _ = ctx[i]
                        agg_i, lps1 = emit_layer_front(l, gms, ft, s0pr)
                        aggs.append(agg_i)
                        ctx[i][3] = lps1
                    for i in range(gn):
                        gms, s0pr, ft, _ = ctx[i]
                        ctx[i][2] = emit_layer_back(l, aggs[i], ft, s0pr)
                for i in range(gn):
                    emit_pool_pair(base + i, ctx[i][2], ctx[i][3])
                base += gn

            # --- readout MLP: relu(x @ Wr1 + br1) @ Wr2 + br2, feature-major ---
            ps_h1 = psph.tile([128, GPC], F32, tag="psh")
            ps_h2 = psph.tile([128, GPC], F32, tag="psh")
            nc.tensor.matmul(ps_h1[:], wr1[:, 0:128],
                             xfm[:], start=True, stop=True)
            nc.tensor.matmul(ps_h2[:], wr1[:, 128:256],
                             xfm[:], start=True, stop=True)
            hid1 = work.tile([128, GPC], BF16, tag="hid1")
            hid2 = work.tile([128, GPC], BF16, tag="hid2")
            nc.vector.tensor_scalar(hid1[:], ps_h1[:], br1a[:], 0.0,
                                    AluOpType.add, AluOpType.max)
            nc.vector.tensor_scalar(hid2[:], ps_h2[:], br1b[:], 0.0,
                                    AluOpType.add, AluOpType.max)
            ps_o = psp1.tile([LAT, GPC], F32, tag="ps1")
            nc.tensor.matmul(ps_o[:], wr2a[:],
                             hid1[:], start=True, stop=False)
            nc.tensor.matmul(ps_o[:], wr2b[:],
                             hid2[:], start=False, stop=True)
            outt = work.tile([LAT, GPC], F32, tag="outt")
            nc.vector.tensor_scalar(outt[:], ps_o[:], br2[:], None, AluOpType.add)
            nc.sync.dma_start(out_ap[:], outt[:])

    nc.compile()
    return nc


def kernel(pos, emb, W_s2n, W1, W2, W3, W4, Ws, Wv, Wr1, br1, Wr2, br2,
           z, batch, edge_index, num_graphs):
    pos = np.asarray(pos, dtype=np.float32)
    z = np.asarray(z)
    emb = np.asarray(emb, dtype=np.float32)
    W_s2n = np.asarray(W_s2n, dtype=np.float32)
    W1 = np.asarray(W1, dtype=np.float32); W2 = np.asarray(W2, dtype=np.float32)
    W3 = np.asarray(W3, dtype=np.float32); W4 = np.asarray(W4, dtype=np.float32)
    Ws = np.asarray(Ws, dtype=np.float32); Wv = np.asarray(Wv, dtype=np.float32)
    Wr1 = np.asarray(Wr1, dtype=np.float32); br1 = np.asarray(br1, dtype=np.float32)
    Wr2 = np.asarray(Wr2, dtype=np.float32); br2 = np.asarray(br2, dtype=np.float32)

    # host prep: embedding lookup folded with input linear
    EW = (emb @ W_s2n) * np.float32(1.0 / np.sqrt(S_MUL))     # [100, 32]
    s0 = EW[z].astype(ml_dtypes.bfloat16)                     # [N, 32]

    # combined transform weights, norm constants folded in
    cs = C_SCALAR * np.float32(1.0 / np.sqrt(S_MUL))
    csb = C_SCALAR * np.float32(INV_SQRT3 / np.sqrt(S_MUL))
    cv = C_VECTOR * np.float32(INV_SQRT3 / np.sqrt(V_MUL))
    wab = np.zeros((3, 224, ND), np.float32)
    for l in range(3):
        wab[l, 0:32, 0:32] = cs * (W1[l] @ Ws[l])
        w3 = cv * (W3[l] @ Wv[l])
        w2 = cv * (W2[l] @ Wv[l])
        for c in range(3):
            wab[l, 32 + 32 * c:48 + 32 * c, 32 + 32 * c:48 + 32 * c] = w3
            wab[l, 112 + 32 * c:144 + 32 * c, 32 + 32 * c:48 + 32 * c] = w2
        wab[l, 208:224, 0:32] = csb * (W4[l] @ Ws[l])
    wab = wab.astype(ml_dtypes.bfloat16)

    # readout weights in padded feature layout: row 32+32c+u <-> v[u, c]
    wr1p = np.zeros((ND, HID), np.float32)
    wr1p[0:32] = Wr1[0:32]
    for c in range(3):
        for u in range(V_MUL):
            wr1p[32 + 32 * c + u] = Wr1[32 + 3 * u + c]
    wr1p = wr1p.astype(ml_dtypes.bfloat16)

    # compact same-graph mask: node i vs its graph's 32 atoms; self excluded
    bdm = np.ones((128, NA), np.float32)
    for i in range(128):
        bdm[i, i % NA] = 0.0
    poolm = np.zeros((128, GPB), np.float32)
    for g in range(GPB):
        poolm[g * NA:(g + 1) * NA, g] = 1.0
    poolm = poolm.astype(ml_dtypes.bfloat16)

    if "nc" not in _CACHE:
        _CACHE["nc"] = _build_program()
    nc = _CACHE["nc"]

    in_maps = []
    for c in range(NCORES):
        psl = pos[c * NPC:(c + 1) * NPC]                       # [8192, 3]
        # posGC[b, 32g+i, 32c+j] = pos of atom j (component c) of node i's graph
        pg = psl.reshape(NBLK, GPB, NA, 3).transpose(0, 1, 3, 2)   # [b, g, c, j]
        posGC = np.ascontiguousarray(
            np.broadcast_to(pg[:, :, None, :, :],
                            (NBLK, GPB, NA, 3, NA)).reshape(NBLK, 128, 96))
        in_maps.append(dict(
            s0=np.ascontiguousarray(s0[c * NPC:(c + 1) * NPC]),
            posGC=posGC,
            posnm=np.ascontiguousarray(psl),
            bd=bdm, wab=wab, poolm=poolm,
            wr1=wr1p, br1=br1.reshape(HID, 1),
            wr2=Wr2.astype(ml_dtypes.bfloat16), br2=br2.reshape(LAT, 1),
        ))

    res = run_bass_kernel_spmd(nc, in_maps, core_ids=list(range(NCORES)))
    out = np.empty((B, LAT), np.float32)
    for c in range(NCORES):
        out[c * GPC:(c + 1) * GPC] = res.results[c]["outfm"].T
    return out
